# revision 25
# baseline (speedup 1.0000x reference)
"""AFT (attention-free-transformer) layer kernel for 8 TRN2 NeuronCores.

Math (the kmax/bmax softmax-stabilizer subtractions cancel exactly in
num/den, and the sigmoid gate folds into the denominator):
    q,k,v = x @ W{q,k,v}.T          (B,N,C), heads (B,H,N,hd)
    eb[h,i,j] = exp(bias[h,i,j]),  bias = rel_pos_table[rel_index]
    num = eb @ (exp(k)*v), den = eb @ exp(k)   (contracted over j)
    out = (num / (den * (1 + exp(-q)))) @ Wp.T + bp

Sharding: output rows i split 8 ways (216 rows/core); everything else
replicated; no collectives. The (H,N,N) bias tensor is materialized on the
host as pure input indexing (rel_pos_table[rel_index] — no arithmetic) and
shipped sharded along i as fp16; the device streams it (the memory-bound
part), computes delta = expm1(bias) in f32 precision (fp16 of exp(bias)
would destroy the small-delta signal), and contracts per head with a
ones-column appended to the moving operand so the same matmul also yields
sum_j m[j,c] (the "+1" part of eb = 1 + delta). All ACT usage is
{exp, copy, identity} — one function-table set, no ACT table reloads.
"""

import sys

for p in ("/opt/trn_rl_repo", "/opt/pypackages"):
    if p not in sys.path:
        sys.path.append(p)

from contextlib import ExitStack

import numpy as np

import concourse.bacc as bacc
import concourse.mybir as mybir
import concourse.tile as tile
from concourse.bass_utils import run_bass_kernel_spmd

B, N, C, H, HD, T = 4, 1728, 128, 8, 16, 12167
NCORES = 8
NI = N // NCORES  # 216 output rows per core
NJT = 14  # j tiles: 13 x 128 + 1 x 64
PJ = [128] * 13 + [64]
F32 = mybir.dt.float32
F16 = mybir.dt.float16
AF = mybir.ActivationFunctionType

_CACHE: dict = {}


def _make_band() -> np.ndarray:
    band = np.zeros((64, B, 288), np.float32)
    for b in range(B):
        for d in range(16):
            k = b * 16 + d
            band[k, b, 112 + k] = 1.0
    return band.reshape(64, B * 288)


def _build_nc():
    nc = bacc.Bacc("TRN2", target_bir_lowering=False, debug=False)

    xT = nc.declare_dram_parameter("xT", [B, C, N], F32, isOutput=False)
    wkv = nc.declare_dram_parameter("wkv", [C, 2 * C], F32, isOutput=False)
    wq = nc.declare_dram_parameter("wq", [C, C], F32, isOutput=False)
    wp = nc.declare_dram_parameter("wp", [C, C], F32, isOutput=False)
    bp = nc.declare_dram_parameter("bp", [C, 1], F32, isOutput=False)
    bt = nc.declare_dram_parameter("biasT", [NJT, 128, NI * H], F16, isOutput=False)
    xq = nc.declare_dram_parameter("xq", [B, C, NI], F32, isOutput=False)
    bandp = nc.declare_dram_parameter("band", [64, B * 288], F32, isOutput=False)
    out = nc.declare_dram_parameter("out", [B, C, NI], F32, isOutput=True)

    with tile.TileContext(nc) as tc, ExitStack() as ctx:
        pool = ctx.enter_context(tc.tile_pool(name="persist", bufs=1))
        stage = ctx.enter_context(tc.tile_pool(name="stage", bufs=2))

        # ---- bias stream: delta = expm1(bias), plus a ones-column ------
        # eb layout per j-tile: [j, i*8+h] for i<NI, then 8 ones at i=NI.
        eb_sb = pool.tile([128, NJT, (NI + 1) * H], F16, tag="eb")
        for jt in range(NJT):
            braw = stage.tile([128, NI * H], F16, tag="braw")
            nc.sync.dma_start(out=braw[:], in_=bt[jt])
            bexp = stage.tile([128, NI * H], F32, tag="bexp")
            nc.scalar.activation(bexp[:], braw[:], AF.Exp)
            nc.vector.tensor_scalar_add(
                eb_sb[:, jt, 0 : NI * H], bexp[:], -1.0
            )
            nc.gpsimd.memset(eb_sb[: PJ[jt], jt, NI * H : (NI + 1) * H], 1.0)

        # ---- load xT / xq, cast to fp16 --------------------------------
        xT_sb = pool.tile([128, B, N], F16, tag="xT")
        for b in range(B):
            xf = stage.tile([128, N], F32, tag="xf32")
            nc.sync.dma_start(out=xf[:], in_=xT[b])
            nc.scalar.activation(xT_sb[:, b, :], xf[:], AF.Copy)
        xq_sb = pool.tile([128, B, NI], F16, tag="xq")
        for b in range(B):
            xqf = stage.tile([128, NI], F32, tag="xqf32")
            nc.sync.dma_start(out=xqf[:], in_=xq[b])
            nc.scalar.activation(xq_sb[:, b, :], xqf[:], AF.Copy)

        # ---- weights + band ---------------------------------------------
        wkv_sb = pool.tile([128, 2 * C], F16, tag="wkv")
        wq_sb = pool.tile([128, C], F16, tag="wq")
        wp_sb = pool.tile([128, C], F16, tag="wp")
        bp_sb = pool.tile([128, 1], F32, tag="bp")
        wf = stage.tile([128, 2 * C], F32, tag="wf32")
        nc.sync.dma_start(out=wf[:], in_=wkv[:])
        nc.scalar.activation(wkv_sb[:], wf[:], AF.Copy)
        wf2 = stage.tile([128, C], F32, tag="wf32b")
        nc.sync.dma_start(out=wf2[:], in_=wq[:])
        nc.scalar.activation(wq_sb[:], wf2[:], AF.Copy)
        wf3 = stage.tile([128, C], F32, tag="wf32c")
        nc.sync.dma_start(out=wf3[:], in_=wp[:])
        nc.scalar.activation(wp_sb[:], wf3[:], AF.Copy)
        nc.sync.dma_start(out=bp_sb[:], in_=bp[:])
        band = pool.tile([64, B, 288], F16, tag="band")
        bandf = stage.tile([64, B * 288], F32, tag="bandf")
        nc.sync.dma_start(out=bandf[:], in_=bandp[:])
        nc.scalar.activation(
            band[:].rearrange("p b c -> p (b c)"), bandf[:], AF.Copy
        )

        # ---- projections ------------------------------------------------
        # M layout per j-tile: [j, h*128 + {0:64 -> ekv (b*16+d), 64:128 -> ek}]
        m_sb = pool.tile([128, NJT, H * 128], F16, tag="m")
        # exp(-q): rows (b%2)*32+d, pair index b//2 on a free dim
        eqsig_sb = pool.tile([64, 2, H, NI], F32, tag="eqsig")
        # repartitioned exp(-q) on rows 64 + b*16 + d (aligned with den)
        eq_hi = pool.tile([128, H, NI], F32, tag="eqhi")

        with tc.tile_pool(name="psum_proj", bufs=2, space="PSUM") as pp:
            for b in range(B):
                for jt in range(NJT):
                    pt = PJ[jt]
                    kv_ps = pp.tile([128, 2 * C], F32, tag="kv")
                    nc.tensor.matmul(
                        kv_ps[:pt],
                        xT_sb[:, b, jt * 128 : jt * 128 + pt],
                        wkv_sb[:],
                        start=True,
                        stop=True,
                    )
                    # ek = exp(k) -> M[., h, 64+b*16+d]
                    m_t = m_sb[:pt, jt, :].rearrange("p (h x) -> p h x", h=H)
                    nc.scalar.activation(
                        m_t[:, :, 64 + b * 16 : 64 + b * 16 + 16],
                        kv_ps[:pt, 0:C].rearrange("p (h d) -> p h d", h=H),
                        AF.Exp,
                    )
                    # ekv = ek * v -> M[., h, b*16+d]
                    nc.vector.tensor_mul(
                        m_t[:, :, b * 16 : b * 16 + 16],
                        m_t[:, :, 64 + b * 16 : 64 + b * 16 + 16],
                        kv_ps[:pt, C : 2 * C].rearrange("p (h d) -> p h d", h=H),
                    )
            # q projection for own rows: per pair of batches, rows (b%2)*32+d
            for h in range(H):
                for pair in range(2):
                    q_ps = pp.tile([64, NI], F32, tag="q")
                    for sb in range(2):
                        b = pair * 2 + sb
                        nc.tensor.matmul(
                            q_ps[sb * 32 : sb * 32 + 16, :],
                            wq_sb[:, h * 16 : h * 16 + 16],
                            xq_sb[:, b, :],
                            start=True,
                            stop=True,
                        )
                        # exp(-q)
                        nc.scalar.activation(
                            eqsig_sb[sb * 32 : sb * 32 + 16, pair, h, :],
                            q_ps[sb * 32 : sb * 32 + 16, :],
                            AF.Exp,
                            scale=-1.0,
                        )
        # repartition exp(-q): rows (b%2)*32+d -> 64 + b*16+d
        for pair in range(2):
            for sb in range(2):
                b = pair * 2 + sb
                nc.sync.dma_start(
                    out=eq_hi[64 + b * 16 : 64 + b * 16 + 16],
                    in_=eqsig_sb[sb * 32 : sb * 32 + 16, pair],
                )

        # ---- main contraction -------------------------------------------
        # acc[:, h, :NI+1]: rows 0:64 = num (b*16+d), rows 64:128 = den;
        # col NI = sum_j m (from the ones-column).
        gg_sb = pool.tile([64, H, NI], F16, tag="gg")
        dhi_sb = pool.tile([128, H, NI], F32, tag="dhi")
        tmp_hi = pool.tile([128, H, NI], F32, tag="tmphi")
        rec_lo = pool.tile([64, H, NI], F32, tag="reclo")
        num_lo = pool.tile([64, H, NI], F32, tag="numlo")
        with tc.tile_pool(name="psum_acc", bufs=1, space="PSUM") as pa:
            acc = pa.tile([128, H, 512], F32, tag="acc")
            for jt in range(NJT):
                pt = PJ[jt]
                eb_t = eb_sb[:pt, jt, :].rearrange("p (i e) -> p i e", e=H)
                for h in range(H):
                    nc.tensor.matmul(
                        acc[:, h, 0 : NI + 1],
                        m_sb[:pt, jt, h * 128 : h * 128 + 128],
                        eb_t[:, :, h : h + 1],
                        start=(jt == 0),
                        stop=(jt == NJT - 1),
                    )
            # num/den = col NI (sum_j m) + cols 0:NI (delta part).
            # D = den * (1 + exp(-q)); r = 1/D; shift r to rows 0:64;
            # gg = num * r.   All batched over heads.
            s_sb = pool.tile([128, H, 1], F32, tag="scol")
            nc.scalar.activation(s_sb[:], acc[:, :, NI : NI + 1], AF.Copy)
            den_d = acc[64:128, :, 0:NI]
            den_s = s_sb[64:128].to_broadcast([64, H, NI])
            nc.vector.tensor_add(dhi_sb[64:128], den_d, den_s)
            nc.vector.tensor_mul(tmp_hi[64:128], dhi_sb[64:128], eq_hi[64:128])
            nc.vector.tensor_add(dhi_sb[64:128], dhi_sb[64:128], tmp_hi[64:128])
            nc.vector.reciprocal(
                dhi_sb[64:128].rearrange("p h f -> p (h f)"),
                dhi_sb[64:128].rearrange("p h f -> p (h f)"),
            )
            nc.sync.dma_start(out=rec_lo[:], in_=dhi_sb[64:128])
            num_s = s_sb[0:64].to_broadcast([64, H, NI])
            nc.vector.tensor_add(num_lo[:], acc[0:64, :, 0:NI], num_s)
            nc.vector.tensor_mul(gg_sb[:], num_lo[:], rec_lo[:])

        # ---- head re-assembly + output projection -----------------------
        with tc.tile_pool(name="psum_fin", bufs=1, space="PSUM") as pf:
            g_ps = pf.tile([128, B, 512], F32, tag="g")
            y_ps = pf.tile([128, B, 512], F32, tag="y")
            for b in range(B):
                for h in range(H):
                    t0 = 112 - (h - b) * 16
                    nc.tensor.matmul(
                        g_ps[:, b, 0:NI],
                        band[:, b, t0 : t0 + 128],
                        gg_sb[:, h, :],
                        start=(h == 0),
                        stop=(h == H - 1),
                    )
                g_sb = stage.tile([128, NI], F16, tag="gsb")
                nc.scalar.activation(g_sb[:], g_ps[:, b, 0:NI], AF.Copy)
                nc.tensor.matmul(
                    y_ps[:, b, 0:NI], wp_sb[:], g_sb[:], start=True, stop=True
                )
                y_sb = stage.tile([128, NI], F32, tag="ysb")
                nc.scalar.activation(
                    y_sb[:], y_ps[:, b, 0:NI], AF.Identity, bias=bp_sb[:]
                )
                nc.sync.dma_start(out=out[b], in_=y_sb[:])

    nc.compile()
    return nc


def _bias_shard(tbl: np.ndarray, ridx: np.ndarray, c: int) -> np.ndarray:
    """Pure input indexing: biasT[jt, j, i*8+h] = tbl[ridx[i_glob, jt*128+j], h]."""
    sl = ridx[c * NI : (c + 1) * NI, :]  # (NI, N)
    vals = tbl[sl]  # (NI, N, H) f32
    vals = vals.transpose(1, 0, 2)  # (N, NI, H)
    padded = np.zeros((NJT * 128, NI, H), tbl.dtype)
    padded[:N] = vals
    return np.ascontiguousarray(
        padded.reshape(NJT, 128, NI * H).astype(np.float16)
    )


def kernel(**inputs: np.ndarray) -> np.ndarray:
    x = np.asarray(inputs["x"], np.float32)
    Wq = np.asarray(inputs["Wq"], np.float32)
    Wk = np.asarray(inputs["Wk"], np.float32)
    Wv = np.asarray(inputs["Wv"], np.float32)
    Wp = np.asarray(inputs["Wp"], np.float32)
    bpv = np.asarray(inputs["bp"], np.float32)
    tbl = np.asarray(inputs["rel_pos_table"], np.float32)
    ridx = np.asarray(inputs["rel_index"], np.int64)

    if "nc" not in _CACHE:
        _CACHE["nc"] = _build_nc()
    nc = _CACHE["nc"]

    xTh = np.ascontiguousarray(x.transpose(0, 2, 1))  # (B, C, N)
    wkvh = np.ascontiguousarray(np.concatenate([Wk.T, Wv.T], axis=1))

    in_maps = []
    for c in range(NCORES):
        in_maps.append(
            {
                "xT": xTh,
                "wkv": wkvh,
                "wq": np.ascontiguousarray(Wq.T),
                "wp": np.ascontiguousarray(Wp.T),
                "bp": np.ascontiguousarray(bpv.reshape(C, 1)),
                "biasT": _bias_shard(tbl, ridx, c),
                "xq": np.ascontiguousarray(xTh[:, :, c * NI : (c + 1) * NI]),
                "band": _make_band(),
            }
        )

    res = run_bass_kernel_spmd(nc, in_maps, core_ids=list(range(NCORES)))
    outs = [r["out"] for r in res.results]  # each (B, C, NI)
    full = np.concatenate(outs, axis=2).transpose(0, 2, 1)
    return np.ascontiguousarray(full.astype(np.float32))
